# revision 2
# baseline (speedup 1.0000x reference)
"""Multi-head causal attention (dense transformer block) on 8 Trainium2 cores.

Sharding: 2-way data parallel over batch x 4-way tensor parallel over heads.
Core c handles batch c//4 and heads 4*(c%4) .. 4*(c%4)+3.

Per-core pipeline (host pre-transposes/packs x and the weight shards once):
  1. QT/KT [hd, t] and V [t, hd] projections from x in fp8e4m3 DoubleRow
     matmuls: both operands are split hi/lo (x = x_hi + x_lo, w = w_hi +
     w_lo, each e4m3) and the product uses the 3-term expansion
     x_hi*w_hi + x_hi*w_lo + x_lo*w_hi. DoubleRow packs 2 contraction
     subtiles per instruction at 0.5 cycles/column, so the projection runs
     at 4/3x the fp32r rate with ~1e-3 relative error. Host pre-scales
     x by 16 and w by 32 to center values in the e4m3 range; the exp scale
     and the output-projection eviction scale fold the factors back out.
  2. Attention per (head, q-chunk) in float32r exactly as before: scores
     computed transposed (S^T [k, q]), exp without max-subtraction, causal
     masking via affine_select on the diagonal tiles, softmax denominators
     via ones-vector matmuls, AV accumulated as out^T [hd, q].
  3. Output projection in 3-term fp8 DoubleRow over head pairs: outT is
     re-quantized hi/lo at normalization time (DVE mul + Pool copy/sub),
     woT arrives packed hi/lo from the host.
  4. ReduceScatter over the 4 cores sharing a batch; host concatenates the
     row shards.
"""

import os
import sys

sys.path.insert(0, "/opt/trn_rl_repo")

import numpy as np
import ml_dtypes

N_CORES = 8
B = 2
T = 2048          # sequence length
D = 2048          # model dim
P = 128           # partitions
HD = 128          # head dim
NHG = 4           # head-groups (cores per batch)
HPC = 4           # heads per core
F = HPC * HD      # 512 per-core q/k/v feature width
TC = 512          # token chunk (matmul free dim)
NTC = T // TC     # 4 token chunks
ND = D // P       # 16 d-subtiles
NJ = ND // 2      # 8 DoubleRow d-pairs
SCALE = float(HD) ** -0.5
XS = 16.0         # host pre-scale on x
WS = 32.0         # host pre-scale on weights
E4 = ml_dtypes.float8_e4m3

# (stationary_row_offset, moving_row_offset) for the 3-term hi/lo product:
# rows 0:2 of a packed tile are the hi pair, rows 2:4 the lo pair.
TERMS = ((0, 0), (2, 0), (0, 2))

_CACHE = {}


def _build(mm_dtype_name: str, reps: int = 1, with_rs: bool = True):
    import concourse.bacc as bacc
    import concourse.mybir as mybir
    import concourse.tile as tile

    dt = mybir.dt
    f32 = dt.float32
    f8 = dt.float8e4
    md = getattr(dt, mm_dtype_name)  # dtype of fp32-path PE-input tiles

    nc = bacc.Bacc(
        "TRN2", target_bir_lowering=False, debug=False, num_devices=N_CORES
    )

    # DoubleRow pair-packed fp8 operands, see _pack_pairs for the layout:
    # [j*128+p, r, m] with r in {hi-A, hi-B, lo-A, lo-B}.
    x8 = nc.dram_tensor("x8", [D // 2, 4, T], f8, kind="ExternalInput")
    wq8 = nc.dram_tensor("wq8", [D // 2, 4, F], f8, kind="ExternalInput")
    wk8 = nc.dram_tensor("wk8", [D // 2, 4, F], f8, kind="ExternalInput")
    wv8 = nc.dram_tensor("wv8", [D // 2, 4, F], f8, kind="ExternalInput")
    wo8 = nc.dram_tensor("wo8", [F // 2, 4, D], f8, kind="ExternalInput")
    out = nc.dram_tensor("out", [T // NHG, D], f32, kind="ExternalOutput")

    with nc.allow_low_precision(reason="fp8 DoubleRow matmul tiles"), \
         tile.TileContext(nc) as tc:
        with (
            tc.tile_pool(name="const", bufs=1) as const,
            tc.tile_pool(name="resident", bufs=1) as res_pool,
            tc.tile_pool(name="dram", bufs=1, space="DRAM") as dram,
        ):
            ones_stage = const.tile([P, P], f32)
            nc.vector.memset(ones_stage[:], 1.0)
            ones_col = const.tile([P, 1], md)
            nc.scalar.copy(ones_col[:], ones_stage[:, 0:1])
            # bc = (1/WS) / den so that outT = (XS*WS/WS) * out_true =
            # XS * out_true, centered for e4m3.
            inv_stage = const.tile([1, P], f32)
            nc.vector.memset(inv_stage[:], 1.0 / WS)
            ones_row = const.tile([1, P], md)
            nc.scalar.copy(ones_row[:], inv_stage[:])

            # ---- resident activation buffers ----
            QT = [res_pool.tile([P, T], md, name=f"QT{h}") for h in range(HPC)]
            KT = [res_pool.tile([P, T], md, name=f"KT{h}") for h in range(HPC)]
            V = [res_pool.tile([P, F], md, name=f"V{i}") for i in range(T // P)]

            bounce = [dram.tile([TC, D], f32, name=f"bounce{qt}")
                      for qt in range(NTC - 1)]
            bounce += [dram.tile([TC // 2, D], f32, name=f"bounce3{hf}")
                       for hf in range(2)]
            rs_out = [dram.tile([TC // NHG, D], f32, name=f"rs_out{qt}")
                      for qt in range(NTC - 1)]
            rs_out += [dram.tile([TC // 2 // NHG, D], f32, name=f"rs_out3{hf}")
                       for hf in range(2)]

            for rep in range(reps):
                _build_body(nc, tc, mybir, md, f32, f8, rep,
                            x8, wq8, wk8, wv8, wo8, out,
                            ones_col, ones_row, QT, KT, V,
                            bounce, rs_out, with_rs)

    nc.compile()
    return nc


def _build_body(nc, tc, mybir, md, f32, f8, rep,
                x8, wq8, wk8, wv8, wo8, out,
                ones_col, ones_row, QT, KT, V,
                bounce, rs_out, with_rs=True):
    DR = mybir.MatmulPerfMode.DoubleRow
    # ---- phase 1: projections (3-term fp8 DoubleRow) ----
    # Two supersteps of 1024 tokens; each loads the packed q/k/v weights
    # once (12 MB of fp8 weight traffic per pass over x).
    TG = 2 * TC
    with tc.tile_pool(name=f"psum1_{rep}", bufs=1, space="PSUM") as psum1, \
         tc.tile_pool(name=f"xw_{rep}", bufs=3) as xw_pool:
        for tg in range(T // TG):
            xts = []
            for j in range(NJ):
                xt = xw_pool.tile(
                    [P, 4, TG], f8, name=f"xt_{rep}_{tg}_{j}", tag="xt",
                    bufs=NJ + 2,
                )
                nc.sync.dma_start(
                    xt[:],
                    x8.ap()[j * P:(j + 1) * P, :, tg * TG:(tg + 1) * TG],
                )
                xts.append(xt)
            wts = {}
            for wname, wP in (("q", wq8), ("k", wk8), ("v", wv8)):
                for j in range(NJ):
                    wt = xw_pool.tile(
                        [P, 4, F], f8, name=f"w{wname}_{rep}_{tg}_{j}",
                        tag="wt", bufs=6,
                    )
                    nc.scalar.dma_start(wt[:], wP.ap()[j * P:(j + 1) * P, :, :])
                    wts[wname, j] = wt
            for wname, dest in (("q", QT), ("k", KT)):
                pss = [
                    psum1.tile(
                        [P, TC], f32, name=f"ps_{wname}{h}{th}_{rep}_{tg}",
                        tag="pq", bufs=8,
                    )
                    for h in range(HPC) for th in range(2)
                ]
                for j in range(NJ):
                    wt = wts[wname, j]
                    for t3, (sr, mr) in enumerate(TERMS):
                        for h in range(HPC):
                            for th in range(2):
                                nc.tensor.matmul(
                                    pss[2 * h + th][:],
                                    wt[:, sr:sr + 2, h * HD:(h + 1) * HD],
                                    xts[j][:, mr:mr + 2,
                                           th * TC:(th + 1) * TC],
                                    start=(j == 0 and t3 == 0),
                                    stop=(j == NJ - 1 and t3 == 2),
                                    perf_mode=DR,
                                )
                for h in range(HPC):
                    for th in range(2):
                        col = tg * TG + th * TC
                        nc.any.tensor_copy(
                            dest[h][:, col:col + TC], pss[2 * h + th][:]
                        )
            pss = [
                psum1.tile(
                    [P, F], f32, name=f"ps_v{ts}_{rep}_{tg}", tag="pq", bufs=8
                )
                for ts in range(TG // P)
            ]
            for j in range(NJ):
                wt = wts["v", j]
                for t3, (sr, mr) in enumerate(TERMS):
                    for ts in range(TG // P):
                        nc.tensor.matmul(
                            pss[ts][:],
                            xts[j][:, sr:sr + 2, ts * P:(ts + 1) * P],
                            wt[:, mr:mr + 2, :],
                            start=(j == 0 and t3 == 0),
                            stop=(j == NJ - 1 and t3 == 2),
                            perf_mode=DR,
                        )
            for ts in range(TG // P):
                nc.any.tensor_copy(V[tg * (TG // P) + ts][:], pss[ts][:])

    # ---- phases 2+3 per q chunk ----
    with tc.tile_pool(name=f"psum2_{rep}", bufs=1, space="PSUM") as psum2, \
         tc.tile_pool(name=f"work_{rep}", bufs=6) as work:
        WOP = []
        for cp in range(HPC // 2):
            row = []
            for etp in range(NTC // 2):
                wo = work.tile([P, 4, 2 * TC], f8,
                               name=f"WOP{rep}_{cp}_{etp}",
                               tag=f"WOP{cp}_{etp}", bufs=1)
                nc.sync.dma_start(
                    wo[:],
                    wo8.ap()[cp * P:(cp + 1) * P, :,
                             etp * 2 * TC:(etp + 1) * 2 * TC],
                )
                row.append(wo)
            WOP.append(row)
        for qt in range(NTC):
            # hi/lo outT per head pair, DoubleRow stationary layout
            OTH = [work.tile([P, 2, TC], f8, name=f"oth{rep}_{qt}_{cp}",
                             tag=f"oth{cp}", bufs=2)
                   for cp in range(HPC // 2)]
            OTL = [work.tile([P, 2, TC], f8, name=f"otl{rep}_{qt}_{cp}",
                             tag=f"otl{cp}", bufs=2)
                   for cp in range(HPC // 2)]
            n_k = (qt + 1) * (TC // P)  # causal: k-subtiles needed
            diag0 = qt * (TC // P)
            korder = list(range(diag0, n_k)) + list(range(diag0))
            SKEW = 2
            for hp in (0, 2):  # head pairs, emission interleaved
                heads = (hp, hp + 1)
                ps_out = {
                    h: psum2.tile(
                        [P, TC], f32, name=f"ps_out{rep}_{qt}_{h}",
                        tag="out", bufs=2,
                    )
                    for h in heads
                }
                ps_den = {
                    h: psum2.tile(
                        [1, TC], f32, name=f"ps_den{rep}_{qt}_{h}",
                        tag="aux", bufs=2,
                    )
                    for h in heads
                }
                pts = {}
                # For diagonal tiles only columns q >= 128*dj are live:
                # S/exp/AV/den all operate on that sub-rectangle (the dead
                # region is never read, so it needs no zeroing), and the
                # causal mask shrinks to one 128x128 triangle block. korder
                # starts at dj=0 (full width), so the start=True matmuls
                # initialize every psum column's has_written bit.
                def live0(kt):
                    # clamp at TC-256: float32r matmuls need >=256 moving
                    # columns for full rate, so narrower is never faster
                    dj = kt - diag0
                    return min(max(0, dj) * P, TC - 2 * P)
                for step in range(n_k + SKEW):
                    if step < n_k:
                        kt = korder[step]
                        c0 = live0(kt)
                        for h in heads:
                            ps_st = psum2.tile(
                                [P, TC], f32,
                                name=f"ps_st{rep}_{qt}_{h}_{kt}",
                                tag="st", bufs=2,
                            )
                            nc.tensor.matmul(
                                ps_st[:, c0:],
                                KT[h][:, kt * P:(kt + 1) * P],
                                QT[h][:, qt * TC + c0:(qt + 1) * TC],
                                start=True,
                                stop=True,
                            )
                            pt = work.tile(
                                [P, TC], md, name=f"pt{rep}_{qt}_{h}_{kt}",
                                tag="pt", bufs=8,
                            )
                            nc.scalar.activation(
                                pt[:, c0:], ps_st[:, c0:],
                                mybir.ActivationFunctionType.Exp,
                                scale=SCALE / (XS * WS) ** 2,
                            )
                            dj = kt - diag0
                            if dj >= 0:
                                # mask [c0, (dj+1)*128): the dead strip below
                                # the triangle plus the triangle block itself
                                me = (dj + 1) * P
                                nc.gpsimd.affine_select(
                                    pt[:, c0:me], pt[:, c0:me],
                                    pattern=[[1, me - c0]],
                                    compare_op=mybir.AluOpType.is_ge,
                                    fill=0.0,
                                    base=-(dj * P - c0),
                                    channel_multiplier=-1,
                                )
                            pts[h, kt] = pt
                    if step >= SKEW:
                        idx = step - SKEW
                        k = korder[idx]
                        c0 = live0(k)
                        for h in heads:
                            nc.tensor.matmul(
                                ps_den[h][:, c0:],
                                ones_col[:],
                                pts[h, k][:, c0:],
                                start=(idx == 0),
                                stop=(idx == n_k - 1),
                            )
                            nc.tensor.matmul(
                                ps_out[h][:, c0:],
                                V[k][:, h * HD:(h + 1) * HD],
                                pts[h, k][:, c0:],
                                start=(idx == 0),
                                stop=(idx == n_k - 1),
                            )
                for h in heads:
                    den = work.tile([1, TC], md, name=f"den{rep}_{qt}_{h}",
                                    tag="den", bufs=2)
                    nc.vector.reciprocal(den[:], ps_den[h][:])
                    ps_bc = psum2.tile(
                        [P, TC], f32, name=f"ps_bc{rep}_{qt}_{h}", tag="aux",
                        bufs=2,
                    )
                    nc.tensor.matmul(
                        ps_bc[:], ones_row[:], den[:],
                        start=True, stop=True,
                    )
                    bc = work.tile([P, TC], f32, name=f"bc{rep}_{qt}_{h}",
                                   tag="bc", bufs=2)
                    nc.any.tensor_copy(bc[:], ps_bc[:])
                    tmp = work.tile([P, TC], f32, name=f"otmp{rep}_{qt}_{h}",
                                    tag="otmp", bufs=2)
                    nc.vector.tensor_mul(tmp[:], ps_out[h][:], bc[:])
                    cp, r = divmod(h, 2)
                    nc.gpsimd.tensor_copy(OTH[cp][:, r, :], tmp[:])
                    nc.gpsimd.tensor_sub(
                        OTL[cp][:, r, :], tmp[:], OTH[cp][:, r, :]
                    )

            # output projection for this q(=t) chunk (resident packed
            # weights, 3-term fp8 DoubleRow over head pairs). ts-outer so
            # bounce rows complete incrementally; the last chunk's
            # reduce-scatter runs in two half-sized pieces so its exposed
            # tail is halved.
            last = qt == NTC - 1
            for ts in range(TC // P):
                for etp in range(NTC // 2):
                    fin = work.tile(
                        [P, 2 * TC], f32, name=f"fin{rep}_{qt}_{ts}_{etp}",
                        tag="fin", bufs=3,
                    )
                    psf = [
                        psum2.tile(
                            [P, TC], f32,
                            name=f"ps_f{rep}_{qt}_{ts}_{etp}_{ee}",
                            tag="f", bufs=2,
                        )
                        for ee in range(2)
                    ]
                    for cp in range(HPC // 2):
                        for t3, (sr, mr) in enumerate(TERMS):
                            for ee in range(2):
                                nc.tensor.matmul(
                                    psf[ee][:],
                                    (OTH if sr == 0 else OTL)[cp][
                                        :, :, ts * P:(ts + 1) * P],
                                    WOP[cp][etp][:, mr:mr + 2,
                                                 ee * TC:(ee + 1) * TC],
                                    start=(cp == 0 and t3 == 0),
                                    stop=(cp == HPC // 2 - 1 and t3 == 2),
                                    perf_mode=DR,
                                )
                    for ee in range(2):
                        nc.any.tensor_scalar_mul(
                            fin[:, ee * TC:(ee + 1) * TC], psf[ee][:],
                            1.0 / (XS * WS),
                        )
                    if last:
                        dst = bounce[NTC - 1 + ts // 2]
                        drow = (ts % 2) * P
                    else:
                        dst = bounce[qt]
                        drow = ts * P
                    nc.sync.dma_start(
                        dst[drow:drow + P,
                            etp * 2 * TC:(etp + 1) * 2 * TC],
                        fin[:],
                    )
                if last and ts % 2 == 1 and with_rs:
                    hf = ts // 2
                    nc.gpsimd.collective_compute(
                        "ReduceScatter",
                        mybir.AluOpType.add,
                        replica_groups=[[0, 1, 2, 3], [4, 5, 6, 7]],
                        ins=[bounce[NTC - 1 + hf].opt()],
                        outs=[rs_out[NTC - 1 + hf].opt()],
                    )
                    rw = TC // 2 // NHG
                    base = qt * (TC // NHG) + hf * rw
                    nc.sync.dma_start(
                        out.ap()[base:base + rw, :],
                        rs_out[NTC - 1 + hf][:],
                    )
            # ---- phase 4: chunked reduce-scatter, overlapped with the
            # next chunk's compute. Core r of each batch group ends up with
            # rows qt*512 + r*128 .. +128; the host interleaves accordingly.
            if not last:
                if with_rs:
                    nc.gpsimd.collective_compute(
                        "ReduceScatter",
                        mybir.AluOpType.add,
                        replica_groups=[[0, 1, 2, 3], [4, 5, 6, 7]],
                        ins=[bounce[qt].opt()],
                        outs=[rs_out[qt].opt()],
                    )
                    nc.sync.dma_start(
                        out.ap()[qt * (TC // NHG):(qt + 1) * (TC // NHG), :],
                        rs_out[qt][:],
                    )
                else:
                    nc.sync.dma_start(
                        out.ap()[qt * (TC // NHG):(qt + 1) * (TC // NHG), :],
                        bounce[qt][0:TC // NHG, :],
                    )
            elif not with_rs:
                for hf in range(2):
                    rw = TC // 2 // NHG
                    base = qt * (TC // NHG) + hf * rw
                    nc.sync.dma_start(
                        out.ap()[base:base + rw, :],
                        bounce[NTC - 1 + hf][0:rw, :],
                    )


def _get_nc():
    name = os.environ.get("ATTN_MM_DTYPE", "float32r")
    reps = int(os.environ.get("ATTN_REPS", "1"))
    key = (name, reps)
    if key not in _CACHE:
        _CACHE[key] = _build(name, reps)
    return _CACHE[key]


last_exec_time_ns = None


def _pack_pairs(aT):
    """[Dc, M] f32 (pre-scaled) -> [Dc//2, 4, M] e4m3 DoubleRow pair-packed.

    out[j*128+p, 0, m] = hi(aT[(2j+0)*128+p, m])   (hi, A-row)
    out[j*128+p, 1, m] = hi(aT[(2j+1)*128+p, m])   (hi, B-row)
    rows 2, 3: same with the lo residual lo = aT - f32(hi).
    """
    Dc, M = aT.shape
    hi = aT.astype(E4)
    lo = (aT - hi.astype(np.float32)).astype(E4)
    h4 = hi.reshape(Dc // 256, 2, P, M)
    l4 = lo.reshape(Dc // 256, 2, P, M)
    packed = np.stack([h4[:, 0], h4[:, 1], l4[:, 0], l4[:, 1]], axis=2)
    return np.ascontiguousarray(packed.reshape(Dc // 2, 4, M))


def make_in_maps(x, w_qkv, w_out):
    x = np.asarray(x, dtype=np.float32)
    w_qkv = np.asarray(w_qkv, dtype=np.float32)
    w_out = np.asarray(w_out, dtype=np.float32)
    x8s = [_pack_pairs(np.ascontiguousarray(x[b].T) * XS) for b in range(B)]
    in_maps = []
    for c in range(N_CORES):
        b, hg = divmod(c, NHG)
        sl = slice(hg * F, (hg + 1) * F)
        in_maps.append({
            "x8": x8s[b],
            "wq8": _pack_pairs(w_qkv[0 * D:1 * D][sl].T * WS),
            "wk8": _pack_pairs(w_qkv[1 * D:2 * D][sl].T * WS),
            "wv8": _pack_pairs(w_qkv[2 * D:3 * D][sl].T * WS),
            "wo8": _pack_pairs(np.ascontiguousarray(w_out[:, sl].T) * WS),
        })
    return in_maps


def kernel(x, w_qkv, w_out):
    import time

    from concourse import bass_utils

    global last_exec_time_ns
    nc = _get_nc()
    in_maps = make_in_maps(x, w_qkv, w_out)

    trace = bool(int(os.environ.get("ATTN_TRACE", "0")))
    res = None
    last_err = None
    for attempt in range(3):
        try:
            res = bass_utils.run_bass_kernel_spmd(
                nc, in_maps, core_ids=list(range(N_CORES)), trace=trace
            )
            break
        except Exception as e:  # transient axon mesh desyncs
            last_err = e
            time.sleep(10 * (attempt + 1))
    if res is None:
        raise last_err
    last_exec_time_ns = res.exec_time_ns

    outs = [res.results[c]["out"] for c in range(N_CORES)]
    # chunked RS layout: core r of a batch group holds, for chunks 0..2,
    # the summed rows qt*TC + r*128 .. +128; for the split last chunk it
    # holds rows 3*TC + hf*256 + r*64 .. +64 for hf in {0, 1}.
    RW = TC // NHG
    full = []
    for b in range(B):
        arr = np.stack(outs[b * NHG:(b + 1) * NHG])      # [r, NTC*RW, D]
        fb = np.empty((T, D), np.float32)
        head = arr[:, :(NTC - 1) * RW].reshape(NHG, NTC - 1, RW, D)
        fb[:(NTC - 1) * TC] = head.transpose(1, 0, 2, 3).reshape(-1, D)
        tail = arr[:, (NTC - 1) * RW:].reshape(NHG, 2, RW // 2, D)
        fb[(NTC - 1) * TC:] = tail.transpose(1, 0, 2, 3).reshape(-1, D)
        full.append(fb)
    return np.stack(full)


# revision 31
# speedup vs baseline: 290.8334x; 290.8334x over previous
"""Multi-head causal attention (dense transformer block) on 8 Trainium2 cores.

Sharding: 2-way data parallel over batch x 4-way tensor parallel over heads.
Core c handles batch c//4 and heads 4*(c%4) .. 4*(c%4)+3.

Per-core pipeline (host pre-transposes/packs x and the weight shards once):
  1. QT/KT [hd, t] and V [t, hd] projections from x in fp8e4m3 DoubleRow
     matmuls: both operands are split hi/lo (x = x_hi + x_lo, w = w_hi +
     w_lo, each e4m3) and the product uses the 3-term expansion
     x_hi*w_hi + x_hi*w_lo + x_lo*w_hi. DoubleRow packs 2 contraction
     subtiles per instruction at 0.5 cycles/column, so the projection runs
     at 4/3x the fp32r rate with ~1e-3 relative error. Host pre-scales
     x by 16 and w by 32 to center values in the e4m3 range; the exp scale
     and the output-projection eviction scale fold the factors back out.
     All DMAs use partition-major packed layouts (one contiguous span per
     partition, 128 descriptors per transfer).
  2. Attention per (head, q-chunk) in float32r: scores computed transposed
     (S^T [k, q]), exp without max-subtraction, causal masking via
     affine_select on the diagonal tiles. The softmax denominator matmul
     uses an all-32 [128,128] stationary so it lands broadcast across all
     PSUM partitions at the same per-column cost: the reciprocal feeds the
     normalization multiply directly (no separate broadcast matmul), and
     the 1/32 folds the outT e4m3 centering scale.
  3. Output projection in 3-term fp8 DoubleRow over head pairs: outT is
     re-quantized hi/lo at normalization time, woT arrives packed hi/lo
     from the host.
  4. ReduceScatter over the 4 cores sharing a batch; host concatenates the
     row shards.

  Schedule: tokens 0..1023 project first (j-outer boot with its own psum
  pool so the PE starts on the first x/w DMA); qt0+qt1 attention (whose
  causal K/V windows live entirely in tokens 0..1023) then interleaves
  with the tg1 projections; output projections and qt2/qt3 follow. One
  8-bank PSUM pool: projections/output share the f ring, attention uses
  st/out/aux.
"""

import os
import sys

sys.path.insert(0, "/opt/trn_rl_repo")

import numpy as np
import ml_dtypes

N_CORES = 8
B = 2
T = 2048          # sequence length
D = 2048          # model dim
P = 128           # partitions
HD = 128          # head dim
NHG = 4           # head-groups (cores per batch)
HPC = 4           # heads per core
F = HPC * HD      # 512 per-core q/k/v feature width
TC = 512          # token chunk (matmul free dim)
NTC = T // TC     # 4 token chunks
ND = D // P       # 16 d-subtiles
NJ = ND // 2      # 8 DoubleRow d-pairs
SCALE = float(HD) ** -0.5
XS = 16.0         # host pre-scale on x
WS = 32.0         # host pre-scale on weights
E4 = ml_dtypes.float8_e4m3

# (stationary_row_offset, moving_row_offset) for the 3-term hi/lo product:
# rows 0:2 of a packed tile are the hi pair, rows 2:4 the lo pair.
TERMS = ((0, 0), (2, 0), (0, 2))

_CACHE = {}


def _build(mm_dtype_name: str, reps: int = 1, with_rs: bool = True):
    import concourse.bacc as bacc
    import concourse.mybir as mybir
    import concourse.tile as tile

    dt = mybir.dt
    f32 = dt.float32
    f8 = dt.float8e4
    md = getattr(dt, mm_dtype_name)  # dtype of fp32-path PE-input tiles

    nc = bacc.Bacc(
        "TRN2", target_bir_lowering=False, debug=False, num_devices=N_CORES
    )

    # DoubleRow pair-packed fp8 operands in partition-major layouts so every
    # DMA is per-partition contiguous (128 descriptors). See _pack_pairs:
    # r in {hi-A, hi-B, lo-A, lo-B} selects the hi/lo x A/B double-row.
    x8 = nc.dram_tensor("x8", [P, NJ, 2, 4, T // 2], f8, kind="ExternalInput")
    wq8 = nc.dram_tensor("wq8", [P, NJ, 4, F], f8, kind="ExternalInput")
    wk8 = nc.dram_tensor("wk8", [P, NJ, 4, F], f8, kind="ExternalInput")
    wv8 = nc.dram_tensor("wv8", [P, NJ, 4, F], f8, kind="ExternalInput")
    wo8 = nc.dram_tensor("wo8", [P, 2, 4, D], f8, kind="ExternalInput")
    out = nc.dram_tensor("out", [T // NHG, D], f32, kind="ExternalOutput")

    with nc.allow_low_precision(reason="fp8 DoubleRow matmul tiles"), \
         tile.TileContext(nc) as tc:
        with (
            tc.tile_pool(name="const", bufs=1) as const,
            tc.tile_pool(name="resident", bufs=1) as res_pool,
            tc.tile_pool(name="dram", bufs=1, space="DRAM") as dram,
        ):
            # All-32 f32r stationary [P, P]: the den matmul then emits
            # WS*den replicated across all 128 output partitions at the
            # same per-column cost, replacing the separate broadcast
            # matmul, and rec = 1/(WS*den) folds the outT e4m3 centering
            # scale for free.
            c32_stage = const.tile([P, P], f32)
            nc.vector.memset(c32_stage[:], WS)
            ones32 = const.tile([P, P], md)
            nc.scalar.copy(ones32[:], c32_stage[:])

            # ---- resident activation buffers ----
            QT = [res_pool.tile([P, T], md, name=f"QT{h}") for h in range(HPC)]
            KT = [res_pool.tile([P, T], md, name=f"KT{h}") for h in range(HPC)]
            V = [res_pool.tile([P, F], md, name=f"V{i}") for i in range(T // P)]

            bounce = [dram.tile([TC, D], f32, name=f"bounce{qt}")
                      for qt in range(NTC - 1)]
            bounce += [dram.tile([TC // 2, D], f32, name=f"bounce3{hf}")
                       for hf in range(2)]
            rs_out = [dram.tile([TC // NHG, D], f32, name=f"rs_out{qt}")
                      for qt in range(NTC - 1)]
            rs_out += [dram.tile([TC // 2 // NHG, D], f32, name=f"rs_out3{hf}")
                       for hf in range(2)]

            for rep in range(reps):
                _build_body(nc, tc, mybir, md, f32, f8, rep,
                            x8, wq8, wk8, wv8, wo8, out,
                            ones32, QT, KT, V,
                            bounce, rs_out, with_rs)

    nc.compile()
    return nc


def _build_body(nc, tc, mybir, md, f32, f8, rep,
                x8, wq8, wk8, wv8, wo8, out,
                ones32, QT, KT, V,
                bounce, rs_out, with_rs=True):
    DR = mybir.MatmulPerfMode.DoubleRow
    TG = 2 * TC
    NTS = TG // P  # token-subtiles per superstep

    # One PSUM pool for everything: projections share the "f" ring with the
    # output-projection psf tiles (2 banks), attention uses st/out/aux
    # (6 banks) -> exactly 8 banks, allowing phase overlap.
    psum = None  # main PSUM pool; opened after the boot pool closes
    with tc.tile_pool(name=f"xw_{rep}", bufs=3) as xw_pool, \
         tc.tile_pool(name=f"work_{rep}", bufs=6) as work:
        WOP = []

        def emit_wo_dma():
            for cp in range(HPC // 2):
                wo = work.tile([P, 4, D], f8, name=f"WOP{rep}_{cp}",
                               tag=f"WOP{cp}", bufs=1)
                nc.sync.dma_start(wo[:], wo8.ap()[:, cp, :, :])
                WOP.append(wo)

        xts, wts = {}, {}

        def emit_xw_dma(tg):
            # ring order must match consumption (q, k, v FIFO); the first
            # x subtile and first q-weight tile head their queues so the
            # boot j-loop's first accumulation starts as early as possible
            for j in range(NJ):
                xt = xw_pool.tile(
                    [P, 4, TG], f8, name=f"xt_{rep}_{tg}_{j}", tag="xt",
                    bufs=NJ,
                )
                nc.sync.dma_start(xt[:], x8.ap()[:, j, tg, :, :])
                xts[tg, j] = xt
                if j == 0:
                    wt = xw_pool.tile(
                        [P, 4, F], f8, name=f"wq_{rep}_{tg}_0",
                        tag="wt", bufs=10,
                    )
                    nc.scalar.dma_start(wt[:], wq8.ap()[:, 0, :, :])
                    wts[tg, "q", 0] = wt
            for wname, wP in (("q", wq8), ("k", wk8), ("v", wv8)):
                for j in range(NJ):
                    if (tg, wname, j) in wts:
                        continue
                    wt = xw_pool.tile(
                        [P, 4, F], f8, name=f"w{wname}_{rep}_{tg}_{j}",
                        tag="wt", bufs=10,
                    )
                    nc.scalar.dma_start(wt[:], wP.ap()[:, j, :, :])
                    wts[tg, wname, j] = wt

        def qk_unit(tg, wname, dest, h, th):
            ps = psum.tile(
                [P, TC], f32, name=f"ps_{wname}{h}{th}_{rep}_{tg}",
                tag="f", bufs=2,
            )
            for j in range(NJ):
                wt = wts[tg, wname, j]
                for t3, (sr, mr) in enumerate(TERMS):
                    nc.tensor.matmul(
                        ps[:],
                        wt[:, sr:sr + 2, h * HD:(h + 1) * HD],
                        xts[tg, j][:, mr:mr + 2, th * TC:(th + 1) * TC],
                        start=(j == 0 and t3 == 0),
                        stop=(j == NJ - 1 and t3 == 2),
                        perf_mode=DR,
                    )
            col = tg * TG + th * TC
            nc.any.tensor_copy(dest[h][:, col:col + TC], ps[:])

        def v_unit(tg, ts):
            ps = psum.tile(
                [P, F], f32, name=f"ps_v{ts}_{rep}_{tg}", tag="f", bufs=2
            )
            for j in range(NJ):
                wt = wts[tg, "v", j]
                for t3, (sr, mr) in enumerate(TERMS):
                    nc.tensor.matmul(
                        ps[:],
                        xts[tg, j][:, sr:sr + 2, ts * P:(ts + 1) * P],
                        wt[:, mr:mr + 2, :],
                        start=(j == 0 and t3 == 0),
                        stop=(j == NJ - 1 and t3 == 2),
                        perf_mode=DR,
                    )
            nc.any.tensor_copy(V[tg * NTS + ts][:], ps[:])

        def proj_units(tg):
            us = []
            for wname, dest in (("q", QT), ("k", KT)):
                for h in range(HPC):
                    for th in range(2):
                        us.append(
                            lambda wname=wname, dest=dest, h=h, th=th:
                            qk_unit(tg, wname, dest, h, th)
                        )
            for ts in range(NTS):
                us.append(lambda ts=ts: v_unit(tg, ts))
            return us

        def alloc_ot(qt):
            OTH = [work.tile([P, 2, TC], f8, name=f"oth{rep}_{qt}_{cp}",
                             tag=f"oth{cp}", bufs=2)
                   for cp in range(HPC // 2)]
            OTL = [work.tile([P, 2, TC], f8, name=f"otl{rep}_{qt}_{cp}",
                             tag=f"otl{cp}", bufs=2)
                   for cp in range(HPC // 2)]
            return OTH, OTL

        def attn_hp(qt, hp, OTH, OTL):
            n_k = (qt + 1) * (TC // P)  # causal: k-subtiles needed
            diag0 = qt * (TC // P)
            korder = list(range(diag0, n_k)) + list(range(diag0))
            SKEW = 2

            def live0(kt):
                # clamp at TC-256: float32r matmuls need >=256 moving
                # columns for full rate, so narrower is never faster
                dj = kt - diag0
                return min(max(0, dj) * P, TC - 2 * P)

            heads = (hp, hp + 1)
            ps_out = {
                h: psum.tile(
                    [P, TC], f32, name=f"ps_out{rep}_{qt}_{h}",
                    tag="out", bufs=2,
                )
                for h in heads
            }
            ps_den = {
                h: psum.tile(
                    [P, TC], f32, name=f"ps_den{rep}_{qt}_{h}",
                    tag="aux", bufs=2,
                )
                for h in heads
            }
            pts = {}
            # For diagonal tiles only columns q >= 128*dj are live:
            # S/exp/AV/den operate on that sub-rectangle (the dead region
            # below the causal triangle is never read), and the causal
            # mask shrinks to one 128x128 triangle block. korder starts
            # at dj=0 (full width), so the start=True matmuls initialize
            # every psum column's has_written bit.
            for step in range(n_k + SKEW + 2):
                if step < n_k:
                    kt = korder[step]
                    c0 = live0(kt)
                    for h in heads:
                        ps_st = psum.tile(
                            [P, TC], f32,
                            name=f"ps_st{rep}_{qt}_{h}_{kt}",
                            tag="st", bufs=2,
                        )
                        nc.tensor.matmul(
                            ps_st[:, c0:],
                            KT[h][:, kt * P:(kt + 1) * P],
                            QT[h][:, qt * TC + c0:(qt + 1) * TC],
                            start=True,
                            stop=True,
                        )
                        pt = work.tile(
                            [P, TC], md, name=f"pt{rep}_{qt}_{h}_{kt}",
                            tag="pt", bufs=6,
                        )
                        nc.scalar.activation(
                            pt[:, c0:], ps_st[:, c0:],
                            mybir.ActivationFunctionType.Exp,
                            scale=SCALE / (XS * WS) ** 2,
                        )
                        dj = kt - diag0
                        if dj >= 0:
                            # mask [c0, (dj+1)*128): the dead strip below
                            # the triangle plus the triangle block itself
                            me = (dj + 1) * P
                            nc.gpsimd.affine_select(
                                pt[:, c0:me], pt[:, c0:me],
                                pattern=[[1, me - c0]],
                                compare_op=mybir.AluOpType.is_ge,
                                fill=0.0,
                                base=-(dj * P - c0),
                                channel_multiplier=-1,
                            )
                        pts[h, kt] = pt
                if SKEW <= step < n_k + SKEW:
                    idx = step - SKEW
                    k = korder[idx]
                    c0 = live0(k)
                    for h in heads:
                        nc.tensor.matmul(
                            ps_den[h][:, c0:],
                            ones32[:],
                            pts[h, k][:, c0:],
                            start=(idx == 0),
                            stop=(idx == n_k - 1),
                        )
                        nc.tensor.matmul(
                            ps_out[h][:, c0:],
                            V[k][:, h * HD:(h + 1) * HD],
                            pts[h, k][:, c0:],
                            start=(idx == 0),
                            stop=(idx == n_k - 1),
                        )

            for h in heads:
                rec = work.tile([P, TC], f32, name=f"rec{rep}_{qt}_{h}",
                                tag="rec", bufs=2)
                nc.vector.reciprocal(rec[:], ps_den[h][:])
                tmp = work.tile([P, TC], f32, name=f"otmp{rep}_{qt}_{h}",
                                tag="otmp", bufs=2)
                nc.vector.tensor_mul(tmp[:], ps_out[h][:], rec[:])
                cp, r = divmod(h, 2)
                nc.vector.tensor_copy(OTH[cp][:, r, :], tmp[:])
                nc.vector.tensor_sub(
                    OTL[cp][:, r, :], tmp[:], OTH[cp][:, r, :]
                )

        def op_chunk(qt, OTH, OTL):
            # output projection for this q(=t) chunk (resident packed
            # weights, 3-term fp8 DoubleRow over head pairs). ts-outer so
            # bounce rows complete incrementally; the last chunk's
            # reduce-scatter runs in two half-sized pieces so its exposed
            # tail is halved.
            last = qt == NTC - 1
            for ts in range(TC // P):
                for etp in range(NTC // 2):
                    fin = work.tile(
                        [P, 2 * TC], f32, name=f"fin{rep}_{qt}_{ts}_{etp}",
                        tag="fin", bufs=3,
                    )
                    # the last chunk may rotate psf into the freed
                    # attention "out" ring to hide eviction latency
                    ptag = "out" if last and (ts * 2 + etp) % 2 else "f"
                    psf = [
                        psum.tile(
                            [P, TC], f32,
                            name=f"ps_f{rep}_{qt}_{ts}_{etp}_{ee}",
                            tag=ptag, bufs=2,
                        )
                        for ee in range(2)
                    ]
                    for cp in range(HPC // 2):
                        for t3, (sr, mr) in enumerate(TERMS):
                            for ee in range(2):
                                nc.tensor.matmul(
                                    psf[ee][:],
                                    (OTH if sr == 0 else OTL)[cp][
                                        :, :, ts * P:(ts + 1) * P],
                                    WOP[cp][:, mr:mr + 2,
                                            (etp * 2 + ee) * TC:
                                            (etp * 2 + ee + 1) * TC],
                                    start=(cp == 0 and t3 == 0),
                                    stop=(cp == HPC // 2 - 1 and t3 == 2),
                                    perf_mode=DR,
                                )
                    for ee in range(2):
                        nc.any.tensor_scalar_mul(
                            fin[:, ee * TC:(ee + 1) * TC], psf[ee][:],
                            1.0 / (XS * WS),
                        )
                    if last:
                        dst = bounce[NTC - 1 + ts // 2]
                        drow = (ts % 2) * P
                    else:
                        dst = bounce[qt]
                        drow = ts * P
                    nc.sync.dma_start(
                        dst[drow:drow + P,
                            etp * 2 * TC:(etp + 1) * 2 * TC],
                        fin[:],
                    )
                if last and ts % 2 == 1 and with_rs:
                    hf = ts // 2
                    nc.gpsimd.collective_compute(
                        "ReduceScatter",
                        mybir.AluOpType.add,
                        replica_groups=[[0, 1, 2, 3], [4, 5, 6, 7]],
                        ins=[bounce[NTC - 1 + hf].opt()],
                        outs=[rs_out[NTC - 1 + hf].opt()],
                    )
                    rw = TC // 2 // NHG
                    base = qt * (TC // NHG) + hf * rw
                    nc.sync.dma_start(
                        out.ap()[base:base + rw, :],
                        rs_out[NTC - 1 + hf][:],
                    )
            # chunked reduce-scatter, overlapped with the next chunk's
            # compute. Core r of each batch group ends up with rows
            # qt*512 + r*128 .. +128; the host interleaves accordingly.
            if not last:
                if with_rs:
                    nc.gpsimd.collective_compute(
                        "ReduceScatter",
                        mybir.AluOpType.add,
                        replica_groups=[[0, 1, 2, 3], [4, 5, 6, 7]],
                        ins=[bounce[qt].opt()],
                        outs=[rs_out[qt].opt()],
                    )
                    nc.sync.dma_start(
                        out.ap()[qt * (TC // NHG):(qt + 1) * (TC // NHG), :],
                        rs_out[qt][:],
                    )
                else:
                    nc.sync.dma_start(
                        out.ap()[qt * (TC // NHG):(qt + 1) * (TC // NHG), :],
                        bounce[qt][0:TC // NHG, :],
                    )
            elif not with_rs:
                for hf in range(2):
                    rw = TC // 2 // NHG
                    base = qt * (TC // NHG) + hf * rw
                    nc.sync.dma_start(
                        out.ap()[base:base + rw, :],
                        bounce[NTC - 1 + hf][0:rw, :],
                    )

        def boot_proj(tg):
            # j-outer with 8 parallel accumulators: the first matmul only
            # needs the first x/w DMA, so the PE starts ~10us earlier than
            # the h-outer form. Runs in its own pool (8 banks) before any
            # attention psum tiles exist.
            with tc.tile_pool(name=f"psboot_{rep}", bufs=1,
                              space="PSUM") as psb:
                for wname, dest in (("q", QT), ("k", KT)):
                    pss = [
                        psb.tile(
                            [P, TC], f32, name=f"psb_{wname}{h}{th}_{rep}",
                            tag="pq", bufs=8,
                        )
                        for h in range(HPC) for th in range(2)
                    ]
                    for j in range(NJ):
                        wt = wts[tg, wname, j]
                        for t3, (sr, mr) in enumerate(TERMS):
                            for h in range(HPC):
                                for th in range(2):
                                    nc.tensor.matmul(
                                        pss[2 * h + th][:],
                                        wt[:, sr:sr + 2,
                                           h * HD:(h + 1) * HD],
                                        xts[tg, j][:, mr:mr + 2,
                                                   th * TC:(th + 1) * TC],
                                        start=(j == 0 and t3 == 0),
                                        stop=(j == NJ - 1 and t3 == 2),
                                        perf_mode=DR,
                                    )
                    for h in range(HPC):
                        for th in range(2):
                            col = tg * TG + th * TC
                            nc.any.tensor_copy(
                                dest[h][:, col:col + TC], pss[2 * h + th][:]
                            )
                pss = [
                    psb.tile(
                        [P, F], f32, name=f"psb_v{ts}_{rep}", tag="pq",
                        bufs=8,
                    )
                    for ts in range(NTS)
                ]
                for j in range(NJ):
                    wt = wts[tg, "v", j]
                    for t3, (sr, mr) in enumerate(TERMS):
                        for ts in range(NTS):
                            nc.tensor.matmul(
                                pss[ts][:],
                                xts[tg, j][:, sr:sr + 2, ts * P:(ts + 1) * P],
                                wt[:, mr:mr + 2, :],
                                start=(j == 0 and t3 == 0),
                                stop=(j == NJ - 1 and t3 == 2),
                                perf_mode=DR,
                            )
                for ts in range(NTS):
                    nc.any.tensor_copy(V[tg * NTS + ts][:], pss[ts][:])

        # ---- schedule ----
        # tg0 projections; then qt0 + qt1 attention (their causal K/V
        # windows live entirely in tokens 0..1023) interleaved with the
        # tg1 projections so score matmuls hide the DoubleRow dispatch
        # and DMA stalls; output projections and qt2/qt3 follow.
        emit_xw_dma(0)
        boot_proj(0)
        emit_xw_dma(1)
        emit_wo_dma()
        psum_cm = tc.tile_pool(name=f"psum_{rep}", bufs=1, space="PSUM")
        psum = psum_cm.__enter__()
        ot0 = alloc_ot(0)
        ot1 = alloc_ot(1)
        attn_hp(0, 0, *ot0)
        attn_hp(0, 2, *ot0)
        attn_hp(1, 0, *ot1)
        units1 = proj_units(1)
        for i, u in enumerate(units1):
            u()
            if i == 11:
                attn_hp(1, 2, *ot1)
        op_chunk(0, *ot0)
        op_chunk(1, *ot1)
        ot2 = alloc_ot(2)
        attn_hp(2, 0, *ot2)
        attn_hp(2, 2, *ot2)
        ot3 = alloc_ot(3)
        attn_hp(3, 0, *ot3)
        op_chunk(2, *ot2)
        attn_hp(3, 2, *ot3)
        op_chunk(3, *ot3)
        psum_cm.__exit__(None, None, None)


def _get_nc():
    name = os.environ.get("ATTN_MM_DTYPE", "float32r")
    reps = int(os.environ.get("ATTN_REPS", "1"))
    key = (name, reps)
    if key not in _CACHE:
        _CACHE[key] = _build(name, reps)
    return _CACHE[key]


last_exec_time_ns = None


def _split4(aT):
    """[Dc, M] f32 -> [Dc//256, 4, P, M] e4m3: per d-pair j, the four
    DoubleRow rows {hi-A, hi-B, lo-A, lo-B} (A = subtile 2j, B = 2j+1)."""
    Dc, M = aT.shape
    hi = aT.astype(E4)
    lo = (aT - hi.astype(np.float32)).astype(E4)
    h4 = hi.reshape(Dc // 256, 2, P, M)
    l4 = lo.reshape(Dc // 256, 2, P, M)
    return np.stack([h4[:, 0], h4[:, 1], l4[:, 0], l4[:, 1]], axis=1)


def _pack_w(aT):
    """[Dc, M] (pre-scaled) -> [P, Dc//256, 4, M] partition-major packed."""
    return np.ascontiguousarray(_split4(aT).transpose(2, 0, 1, 3))


def _pack_x(aT):
    """[D, T] (pre-scaled) -> [P, NJ, 2, 4, T//2]: (p, j, tg, r, t')."""
    s = _split4(aT)                       # [NJ, 4, P, T]
    s = s.reshape(NJ, 4, P, 2, T // 2)    # split T into supersteps
    return np.ascontiguousarray(s.transpose(2, 0, 3, 1, 4))


def make_in_maps(x, w_qkv, w_out):
    x = np.asarray(x, dtype=np.float32)
    w_qkv = np.asarray(w_qkv, dtype=np.float32)
    w_out = np.asarray(w_out, dtype=np.float32)
    x8s = [_pack_x(np.ascontiguousarray(x[b].T) * XS) for b in range(B)]
    in_maps = []
    for c in range(N_CORES):
        b, hg = divmod(c, NHG)
        sl = slice(hg * F, (hg + 1) * F)
        in_maps.append({
            "x8": x8s[b],
            "wq8": _pack_w(w_qkv[0 * D:1 * D][sl].T * WS),
            "wk8": _pack_w(w_qkv[1 * D:2 * D][sl].T * WS),
            "wv8": _pack_w(w_qkv[2 * D:3 * D][sl].T * WS),
            "wo8": _pack_w(np.ascontiguousarray(w_out[:, sl].T) * WS),
        })
    return in_maps


def kernel(x, w_qkv, w_out):
    import time

    from concourse import bass_utils

    global last_exec_time_ns
    nc = _get_nc()
    in_maps = make_in_maps(x, w_qkv, w_out)

    trace = bool(int(os.environ.get("ATTN_TRACE", "0")))
    res = None
    last_err = None
    for attempt in range(3):
        try:
            res = bass_utils.run_bass_kernel_spmd(
                nc, in_maps, core_ids=list(range(N_CORES)), trace=trace
            )
            break
        except Exception as e:  # transient axon mesh desyncs
            last_err = e
            time.sleep(10 * (attempt + 1))
    if res is None:
        raise last_err
    last_exec_time_ns = res.exec_time_ns

    outs = [res.results[c]["out"] for c in range(N_CORES)]
    # chunked RS layout: core r of a batch group holds, for chunks 0..2,
    # the summed rows qt*TC + r*128 .. +128; for the split last chunk it
    # holds rows 3*TC + hf*256 + r*64 .. +64 for hf in {0, 1}.
    RW = TC // NHG
    full = []
    for b in range(B):
        arr = np.stack(outs[b * NHG:(b + 1) * NHG])      # [r, NTC*RW, D]
        fb = np.empty((T, D), np.float32)
        head = arr[:, :(NTC - 1) * RW].reshape(NHG, NTC - 1, RW, D)
        fb[:(NTC - 1) * TC] = head.transpose(1, 0, 2, 3).reshape(-1, D)
        tail = arr[:, (NTC - 1) * RW:].reshape(NHG, 2, RW // 2, D)
        fb[(NTC - 1) * TC:] = tail.transpose(1, 0, 2, 3).reshape(-1, D)
        full.append(fb)
    return np.stack(full)
